# revision 29
# baseline (speedup 1.0000x reference)
"""Trainium2 8-core tensor-parallel GQA attention kernel (Bass/Tile).

Problem: B=1, S=2048, D=4096, H=32 query heads, Hk=8 kv heads, hd=128,
RoPE + causal mask + KV-cache identity scatter + output projection.

Sharding (8 cores): head-parallel tensor parallel.
  - core c: query heads [4c..4c+4), kv head c
  - wq/wk/wv column-sharded, wo column-sharded (AllGather of attention
    outputs instead of AllReduce of wo partials: the wo matmul performs
    the cross-head reduction locally after an AllGather of O^T, which
    moves 8x fewer bytes than an AllReduce of wo partials).

Layout: activations are kept transposed ("feature-major") on chip:
  xT [D, S], qT/kT [hd, S], scores^T [sk, sq] so that softmax's
  key-reduction maps to a ones-matmul and no transposes are needed in
  the attention inner loop.  RoPE's even/odd pairs are made contiguous
  by permuting the columns of wq/wk host-side (even hd indices first);
  the permutation cancels in q.k and is undone host-side for cache_k.

Compute dtype: bf16 (fp32 PSUM accumulation, fp32 softmax denominators).
"""
import numpy as np
import ml_dtypes

# ---- problem constants (hardcoded per spec) ----
S = 2048
D = 4096
H = 32
HK = 8
HD = 128
NCORES = 8
HPC = H // NCORES          # 4 query heads per core
QW = HPC * HD              # 512 q/wo columns per core
CHUNK = 512                # sq chunk
NCH = S // CHUNK           # 4 chunks
NDT = D // 128             # 32 d-tiles
NKT = S // 128             # 16 sk tiles
SCALE = float(HD) ** -0.5
BF = ml_dtypes.bfloat16

# even hd indices first, then odd (RoPE pair trick)
EVEN_FIRST = np.concatenate([np.arange(0, HD, 2), np.arange(1, HD, 2)])

_CACHE = {}
PHASE_MARKS = []


def _mark(nc, label):
    nid = nc.next_id()   # consumes one id; records emission position
    PHASE_MARKS.append((nid, label))


def phase_of(inst_name):
    try:
        n = int(inst_name.split("-")[1])
    except Exception:
        return "?"
    lab = "init"
    for nid, l in PHASE_MARKS:
        if n >= nid:
            lab = l
        else:
            break
    return lab


def _build_nc():
    import concourse.bacc as bacc
    import concourse.mybir as mybir
    import concourse.tile as tile

    BF16 = mybir.dt.bfloat16
    F32 = mybir.dt.float32
    AF = mybir.ActivationFunctionType
    ALU = mybir.AluOpType

    nc = bacc.Bacc("TRN2", target_bir_lowering=False, debug=False, num_devices=NCORES)

    # ---- per-core external inputs ----
    xT = nc.dram_tensor("xT", [D, S], BF16, kind="ExternalInput").ap()
    wq = nc.dram_tensor("wq", [D, QW], BF16, kind="ExternalInput").ap()    # col-permuted
    wk = nc.dram_tensor("wk", [D, HD], BF16, kind="ExternalInput").ap()    # col-permuted
    wv = nc.dram_tensor("wv", [D, HD], BF16, kind="ExternalInput").ap()
    wo = nc.dram_tensor("wo", [D, QW], BF16, kind="ExternalInput").ap()    # col slice
    cosT = nc.dram_tensor("cosT", [HD // 2, S], BF16, kind="ExternalInput").ap()
    sinT = nc.dram_tensor("sinT", [HD // 2, S], BF16, kind="ExternalInput").ap()
    maskd = nc.dram_tensor("maskd", [S, CHUNK], F32, kind="ExternalInput").ap()  # diag blocks, prescaled
    ident = nc.dram_tensor("ident", [128, 128], BF16, kind="ExternalInput").ap()

    # ---- per-core external outputs ----
    yt_o = nc.dram_tensor("yt", [QW, S], F32, kind="ExternalOutput").ap()   # y^T rows [c*512, (c+1)*512)
    kT_o = nc.dram_tensor("kT", [HD, S], BF16, kind="ExternalOutput").ap()  # roped k^T (hd permuted)
    v_o = nc.dram_tensor("v", [S, HD], BF16, kind="ExternalOutput").ap()    # v natural

    with tile.TileContext(nc) as tc:
        with (
            tc.tile_pool(name="persist", bufs=1) as persist,
            tc.tile_pool(name="xt", bufs=5) as xtp,
            tc.tile_pool(name="qt", bufs=9) as qtp,
            tc.tile_pool(name="rope", bufs=2) as ropep,
            tc.tile_pool(name="vt", bufs=2) as vtp,
            tc.tile_pool(name="md", bufs=4) as mdp,
            tc.tile_pool(name="recb", bufs=2) as recbp,
            tc.tile_pool(name="on", bufs=3) as onp,
            tc.tile_pool(name="ys", bufs=2) as ysp,
            tc.tile_pool(name="pt", bufs=4) as ptp,
            tc.tile_pool(name="og", bufs=2) as ogp,
            tc.tile_pool(name="small", bufs=4) as small,
            tc.tile_pool(name="psum", bufs=8, space="PSUM") as psum,
            tc.tile_pool(name="dram", bufs=1, space="DRAM") as dram,
        ):
            # ---- persistent tiles ----
            idt = persist.tile([128, 128], BF16, tag="ident")
            nc.sync.dma_start(idt[:], ident[:])
            cos_sb = persist.tile([HD // 2, S], BF16, tag="cos")
            sin_sb = persist.tile([HD // 2, S], BF16, tag="sin")
            nc.sync.dma_start(cos_sb[:], cosT[:])
            nc.sync.dma_start(sin_sb[:], sinT[:])
            ones = persist.tile([128, 1], BF16, tag="ones")
            nc.vector.memset(ones[:], 1.0)
            ones_row = persist.tile([1, 128], BF16, tag="ones_row")
            nc.vector.memset(ones_row[:], 1.0)

            # resident weights, [128, NDT*cols] with d-tile-major columns.
            # Split per d-tile so the first matmuls don't wait for the
            # whole preload; wq/wk/wv first (stage P), wo last (stage W).
            wqh_sb = [persist.tile([128, NDT * HD], BF16, tag=f"wq{h}", name=f"wqh{h}")
                      for h in range(HPC)]
            wk_sb = persist.tile([128, NDT * HD], BF16, tag="wk")
            wv_sb = persist.tile([128, NDT * HD], BF16, tag="wv")
            wo_sb = persist.tile([128, NDT * QW], BF16, tag="wo")

            def load_wqh(h, split=False):
                if split:
                    hw_ = NDT // 2
                    for piece in range(2):
                        nc.sync.dma_start(
                            wqh_sb[h][:, piece * hw_ * HD:(piece + 1) * hw_ * HD]
                            .rearrange("p (d q) -> p d q", d=hw_),
                            wq[piece * hw_ * 128:(piece + 1) * hw_ * 128, h * HD:(h + 1) * HD]
                            .rearrange("(d p) q -> p d q", p=128))
                else:
                    nc.sync.dma_start(
                        wqh_sb[h][:].rearrange("p (d q) -> p d q", d=NDT),
                        wq[:, h * HD:(h + 1) * HD].rearrange("(d p) q -> p d q", p=128))

            def load_wkv():
                nc.sync.dma_start(wk_sb[:].rearrange("p (d q) -> p d q", d=NDT),
                                  wk.rearrange("(d p) q -> p d q", p=128))
                nc.sync.dma_start(wv_sb[:].rearrange("p (d q) -> p d q", d=NDT),
                                  wv.rearrange("(d p) q -> p d q", p=128))

            def load_wo():
                nc.sync.dma_start(wo_sb[:].rearrange("p (d q) -> p d q", d=NDT),
                                  wo.rearrange("(d p) q -> p d q", p=128))

            load_wqh(0, split=True)

            # persistent activations
            kTr = persist.tile([128, S], BF16, tag="kTr")
            vnat = persist.tile([128, S], BF16, tag="vnat")       # sk-tile t at cols [t*128, ..)

            ag_ins = []
            ag_outs = []
            qt_chunks = []
            md_chunks = []

            def rope(dst, dst_col, ps, j):
                """dst[:, dst_col:dst_col+CHUNK] = rope(ps) (bf16 out).

                ps: psum [128, CHUNK] f32, rows 0:64 = even pairs (x0),
                rows 64:128 = odd (x1)."""
                c_sl = cos_sb[:, j * CHUNK:(j + 1) * CHUNK]
                s_sl = sin_sb[:, j * CHUNK:(j + 1) * CHUNK]
                x0 = ps[0:64, :]
                x1 = ps[64:128, :]
                t0 = ropep.tile([64, CHUNK], F32, tag="rt0")
                t1 = ropep.tile([64, CHUNK], F32, tag="rt1")
                nc.vector.tensor_tensor(t0[:], x0, c_sl, op=ALU.mult)
                nc.vector.tensor_tensor(t1[:], x1, s_sl, op=ALU.mult)
                nc.vector.tensor_tensor(dst[0:64, dst_col:dst_col + CHUNK], t0[:], t1[:], op=ALU.subtract)
                t2 = ropep.tile([64, CHUNK], F32, tag="rt0")
                t3 = ropep.tile([64, CHUNK], F32, tag="rt1")
                nc.vector.tensor_tensor(t2[:], x0, s_sl, op=ALU.mult)
                nc.vector.tensor_tensor(t3[:], x1, c_sl, op=ALU.mult)
                nc.vector.tensor_tensor(dst[64:128, dst_col:dst_col + CHUNK], t2[:], t3[:], op=ALU.add)

            def load_x(j):
                c0 = j * CHUNK
                xsup = []
                for g in range(NDT // 8):
                    xs = xtp.tile([128, 8 * CHUNK], BF16, tag="xt", name=f"xs{j}_{g}")
                    nc.sync.dma_start(
                        xs[:].rearrange("p (d q) -> p d q", d=8),
                        xT[g * 1024:(g + 1) * 1024, c0:c0 + CHUNK].rearrange("(d p) q -> p d q", p=128))
                    xsup.append(xs)
                return xsup

            def stage_P(j, xsup, first=False):
                _mark(nc, f"P{j}")
                c0 = j * CHUNK

                def xts(d):
                    return xsup[d // 8][:, (d % 8) * CHUNK:(d % 8 + 1) * CHUNK]

                mds = []
                for jj in range(4):
                    md = mdp.tile([128, CHUNK], F32, tag="md", name=f"md{j}_{jj}")
                    nc.sync.dma_start(md[:], maskd[c0 + jj * 128: c0 + (jj + 1) * 128, :])
                    mds.append(md)
                md_chunks.append(mds)
                if first:
                    for _h in range(1, HPC):
                        load_wqh(_h)
                    load_wkv()
                    load_wo()
                qts = []
                for h in range(HPC):
                    qps = psum.tile([128, CHUNK], F32, tag="m", name=f"qps{j}_{h}")
                    for d in range(NDT):
                        nc.tensor.matmul(qps[:], wqh_sb[h][:, d * HD:(d + 1) * HD],
                                         xts(d), start=(d == 0), stop=(d == NDT - 1))
                    qt = qtp.tile([128, CHUNK], BF16, tag="qt", name=f"qt{j}_{h}")
                    rope(qt, 0, qps, j)
                    qts.append(qt)
                qt_chunks.append(qts)
                kps = psum.tile([128, CHUNK], F32, tag="m")
                vps = psum.tile([128, CHUNK], F32, tag="m")
                for d in range(NDT):
                    st_, sp_ = (d == 0), (d == NDT - 1)
                    nc.tensor.matmul(kps[:], wk_sb[:, d * HD:(d + 1) * HD], xts(d),
                                     start=st_, stop=sp_)
                    nc.tensor.matmul(vps[:], wv_sb[:, d * HD:(d + 1) * HD], xts(d),
                                     start=st_, stop=sp_)
                rope(kTr, c0, kps, j)
                nc.sync.dma_start(kT_o[:, c0:c0 + CHUNK], kTr[:, c0:c0 + CHUNK])
                vt = vtp.tile([128, CHUNK], BF16, tag="vt")
                nc.vector.tensor_copy(vt[:], vps[:])
                for tt in range(CHUNK // 128):
                    tp = psum.tile([128, 128], BF16, tag="m", name=f"tp{j}_{tt}")
                    nc.tensor.transpose(tp[:], vt[:, tt * 128:(tt + 1) * 128], idt[:])
                    t_glob = j * (CHUNK // 128) + tt
                    nc.scalar.activation(vnat[:, t_glob * 128:(t_glob + 1) * 128], tp[:], AF.Copy)
                    nc.sync.dma_start(v_o[t_glob * 128:(t_glob + 1) * 128, :],
                                      vnat[:, t_glob * 128:(t_glob + 1) * 128])

            def stage_A(j):
                _mark(nc, f"A{j}")
                c0 = j * CHUNK
                nblk = 4 * (j + 1)
                mds = md_chunks[j]
                ag_in = dram.tile([QW, CHUNK], BF16, tag=f"agin{j}")

                def do_norm(den, ov, h):
                    rec = small.tile([1, CHUNK], F32, tag="rec")
                    nc.vector.reciprocal(rec[:], den[:])
                    rec_d = dram.tile([1, CHUNK], F32, tag=f"recd{h % 2}", name=f"recd{j}_{h}")
                    nc.sync.dma_start(rec_d[:], rec[:])
                    recb = recbp.tile([128, CHUNK], F32, tag="recb")
                    nc.sync.dma_start(recb[:], rec_d[0:1, :].partition_broadcast(128))
                    onorm = onp.tile([128, CHUNK], BF16, tag="onorm")
                    nc.vector.tensor_tensor(onorm[:], ov[:], recb[:], op=ALU.mult)
                    nc.sync.dma_start(ag_in[h * 128:(h + 1) * 128, :], onorm[:])

                parts = []

                def issue_half(pi):
                    ag_o = dram.tile([NCORES * 2 * HD, CHUNK], BF16, tag=f"agout{j}_{pi}",
                                     addr_space="Shared", name=f"agout{j}_{pi}")
                    nc.gpsimd.collective_compute(
                        "AllGather", ALU.bypass,
                        ins=[ag_in[pi * 2 * HD:(pi + 1) * 2 * HD, :].opt()], outs=[ag_o.opt()],
                        replica_groups=[list(range(NCORES))],
                    )
                    parts.append(ag_o)

                pending = []
                for h in range(HPC):
                    q_sl = qt_chunks[j][h][:]
                    den = psum.tile([1, CHUNK], F32, tag="m")
                    ov = psum.tile([128, CHUNK], F32, tag="m")
                    queue = []      # (t, pt) awaiting den/ov matmuls (depth 2)
                    for t in range(nblk):
                        st = psum.tile([128, CHUNK], F32, tag="m")
                        nc.tensor.matmul(st[:], kTr[:, t * 128:(t + 1) * 128], q_sl,
                                         start=True, stop=True)
                        if t == 0 and pending:
                            do_norm(*pending.pop(0))
                        if t == 1 and h == 2 and j == NCH - 1:
                            issue_half(0)
                        if t >= nblk - 4:
                            nc.vector.tensor_tensor(st[:], st[:], mds[t - (nblk - 4)][:], op=ALU.add)
                        pt = ptp.tile([128, CHUNK], BF16, tag="pt")
                        nc.scalar.activation(pt[:], st[:], AF.Exp, scale=SCALE)
                        queue.append((t, pt))
                        if len(queue) > 2:
                            tp_, pv = queue.pop(0)
                            nc.tensor.matmul(den[:], ones[:, 0:1], pv[:],
                                             start=(tp_ == 0), stop=False)
                            nc.tensor.matmul(ov[:], vnat[:, tp_ * 128:(tp_ + 1) * 128], pv[:],
                                             start=(tp_ == 0), stop=False)
                    while queue:
                        tp_, pv = queue.pop(0)
                        nc.tensor.matmul(den[:], ones[:, 0:1], pv[:],
                                         start=(tp_ == 0), stop=(tp_ == nblk - 1))
                        nc.tensor.matmul(ov[:], vnat[:, tp_ * 128:(tp_ + 1) * 128], pv[:],
                                         start=(tp_ == 0), stop=(tp_ == nblk - 1))
                    pending.append((den, ov, h))
                while pending:
                    do_norm(*pending.pop(0))
                if j == NCH - 1:
                    issue_half(1)
                else:
                    ag_o = dram.tile([H * HD, CHUNK], BF16, tag=f"agout{j}",
                                     addr_space="Shared", name=f"agoutw{j}")
                    nc.gpsimd.collective_compute(
                        "AllGather", ALU.bypass,
                        ins=[ag_in.opt()], outs=[ag_o.opt()],
                        replica_groups=[list(range(NCORES))],
                    )
                    parts.append(ag_o)
                ag_outs.append(tuple(parts))
                ag_ins.append(ag_in)

            def stage_W(j):
                _mark(nc, f"W{j}")
                c0 = j * CHUNK
                ag_parts = ag_outs[j]
                yps = [psum.tile([128, CHUNK], F32, tag="m", name=f"yps{j}_{_d}") for _d in range(QW // 128)]
                # whole AG: rows are e = head*128 directly.
                # split AG part pi: heads {4r+2pi, 4r+2pi+1} at rows r*256 -> e = 4r+2pi+hl
                ogsup = []
                es_all = []
                if len(ag_parts) == 1:
                    for g in range(NDT // 8):
                        ogs = ogp.tile([128, 8 * CHUNK], BF16, tag="og", name=f"og{j}_{g}")
                        nc.sync.dma_start(
                            ogs[:].rearrange("p (d q) -> p d q", d=8),
                            ag_parts[0][g * 1024:(g + 1) * 1024, :].rearrange("(d p) q -> p d q", p=128))
                        ogsup.append(ogs)
                        es_all.append([8 * g + i for i in range(8)])
                else:
                    for pi, ag in enumerate(ag_parts):
                        for g in range(2):
                            ogs = ogp.tile([128, 8 * CHUNK], BF16, tag="og", name=f"og{j}_{pi}_{g}")
                            nc.sync.dma_start(
                                ogs[:].rearrange("p (d q) -> p d q", d=8),
                                ag[g * 1024:(g + 1) * 1024, :].rearrange("(d p) q -> p d q", p=128))
                            ogsup.append(ogs)
                            rs = [4 * (4 * g + rr) + 2 * pi + hl for rr in range(4) for hl in range(2)]
                            es_all.append(rs)
                for gi, ogs in enumerate(ogsup):
                    for i, e in enumerate(es_all[gi]):
                        og = ogs[:, i * CHUNK:(i + 1) * CHUNK]
                        first = (gi == 0 and i == 0)
                        last = (gi == len(ogsup) - 1 and i == len(es_all[gi]) - 1)
                        for dt_ in range(QW // 128):
                            nc.tensor.matmul(
                                yps[dt_][:], wo_sb[:, e * QW + dt_ * 128: e * QW + (dt_ + 1) * 128],
                                og, start=first, stop=last)
                for dt_ in range(QW // 128):
                    ysb = ysp.tile([128, CHUNK], F32, tag="ysb")
                    nc.scalar.activation(ysb[:], yps[dt_][:], AF.Copy)
                    nc.sync.dma_start(yt_o[dt_ * 128:(dt_ + 1) * 128, c0:c0 + CHUNK], ysb[:])

            # ---- emission: pipeline P/A, W trails by one chunk.
            # Weight preloads are staggered so early matmuls aren't stuck
            # behind the full 37MB preload in the DMA queues.
            warm_in = dram.tile([128, 16], BF16, tag="warm_in")
            warm_out = dram.tile([NCORES * 128, 16], BF16, tag="warm_out", addr_space="Shared")
            nc.gpsimd.collective_compute(
                "AllGather", ALU.bypass,
                ins=[warm_in.opt()], outs=[warm_out.opt()],
                replica_groups=[list(range(NCORES))],
            )
            xs_next = load_x(0)
            for j in range(NCH):
                stage_P(j, xs_next, first=(j == 0))
                if j + 1 < NCH:
                    xs_next = load_x(j + 1)
                stage_A(j)
                if j == 2:
                    stage_W(0)
                    stage_W(1)
            stage_W(2)
            stage_W(3)

    nc.compile()
    return nc


def _get_nc():
    if "nc" not in _CACHE:
        _CACHE["nc"] = _build_nc()
    return _CACHE["nc"]


def _prep_in_maps(x, freqs_cos, freqs_sin, mask, wq, wk, wv, wo):
    xT = np.ascontiguousarray(x.reshape(S, D).T).astype(BF)
    cosT = np.ascontiguousarray(freqs_cos.T).astype(BF)
    sinT = np.ascontiguousarray(freqs_sin.T).astype(BF)
    # stacked diagonal 512x512 blocks of mask^T, prescaled by 1/SCALE
    maskT = np.ascontiguousarray(mask.T).astype(np.float32)
    maskd = np.concatenate(
        [maskT[j * CHUNK:(j + 1) * CHUNK, j * CHUNK:(j + 1) * CHUNK] for j in range(NCH)],
        axis=0) * (1.0 / SCALE)
    maskd = maskd.astype(np.float32)
    ident = np.eye(128, dtype=BF)

    wqp = wq.reshape(D, H, HD)[:, :, EVEN_FIRST].reshape(D, H * HD).astype(BF)
    wkp = wk.reshape(D, HK, HD)[:, :, EVEN_FIRST].reshape(D, HK * HD).astype(BF)
    wv_ = wv.astype(BF)
    wo_ = wo.astype(BF)

    in_maps = []
    for c in range(NCORES):
        in_maps.append({
            "xT": xT,
            "wq": np.ascontiguousarray(wqp[:, c * QW:(c + 1) * QW]),
            "wk": np.ascontiguousarray(wkp[:, c * HD:(c + 1) * HD]),
            "wv": np.ascontiguousarray(wv_[:, c * HD:(c + 1) * HD]),
            "wo": np.ascontiguousarray(wo_[:, c * QW:(c + 1) * QW]),
            "cosT": cosT,
            "sinT": sinT,
            "maskd": maskd,
            "ident": ident,
        })
    return in_maps


def _assemble(results):
    # y: [1, S, D]
    ytT = np.concatenate([results[c]["yt"] for c in range(NCORES)], axis=0)  # [D, S]
    y = np.ascontiguousarray(ytT.T)[None].astype(np.float32)
    # cache_k: [1, S, HK, HD]; kT per core is [HD(perm), S]
    cache_k = np.empty((1, S, HK, HD), np.float32)
    cache_v = np.empty((1, S, HK, HD), np.float32)
    for c in range(NCORES):
        k_perm = results[c]["kT"].astype(np.float32).T     # [S, HD] permuted cols
        k_nat = np.empty((S, HD), np.float32)
        k_nat[:, EVEN_FIRST] = k_perm
        cache_k[0, :, c, :] = k_nat
        cache_v[0, :, c, :] = results[c]["v"].astype(np.float32)
    return y, cache_k, cache_v


def run_on_hw(in_maps, trace=False):
    from concourse.bass_utils import run_bass_kernel_spmd
    nc = _get_nc()
    return run_bass_kernel_spmd(nc, in_maps, core_ids=list(range(NCORES)), trace=trace)


def kernel(x, freqs_cos, freqs_sin, mask, input_idexes, cache_k, cache_v, wq, wk, wv, wo):
    idx = np.asarray(input_idexes)
    assert np.array_equal(idx, np.arange(S)), "kernel assumes identity cache scatter"
    in_maps = _prep_in_maps(np.asarray(x), np.asarray(freqs_cos), np.asarray(freqs_sin),
                            np.asarray(mask), np.asarray(wq), np.asarray(wk),
                            np.asarray(wv), np.asarray(wo))
    res = run_on_hw(in_maps, trace=False)
    return _assemble(res.results)


# revision 30
# speedup vs baseline: 1.0502x; 1.0502x over previous
"""Trainium2 8-core tensor-parallel GQA attention kernel (Bass/Tile).

Problem: B=1, S=2048, D=4096, H=32 query heads, Hk=8 kv heads, hd=128,
RoPE + causal mask + KV-cache identity scatter + output projection.

Sharding (8 cores): head-parallel tensor parallel.
  - core c: query heads [4c..4c+4), kv head c
  - wq/wk/wv column-sharded, wo column-sharded (AllGather of attention
    outputs instead of AllReduce of wo partials: the wo matmul performs
    the cross-head reduction locally after an AllGather of O^T, which
    moves 8x fewer bytes than an AllReduce of wo partials).

Layout: activations are kept transposed ("feature-major") on chip:
  xT [D, S], qT/kT [hd, S], scores^T [sk, sq] so that softmax's
  key-reduction maps to a ones-matmul and no transposes are needed in
  the attention inner loop.  RoPE's even/odd pairs are made contiguous
  by permuting the columns of wq/wk host-side (even hd indices first);
  the permutation cancels in q.k and is undone host-side for cache_k.

Compute dtype: bf16 (fp32 PSUM accumulation, fp32 softmax denominators).
"""
import numpy as np
import ml_dtypes

# ---- problem constants (hardcoded per spec) ----
S = 2048
D = 4096
H = 32
HK = 8
HD = 128
NCORES = 8
HPC = H // NCORES          # 4 query heads per core
QW = HPC * HD              # 512 q/wo columns per core
CHUNK = 512                # sq chunk
NCH = S // CHUNK           # 4 chunks
NDT = D // 128             # 32 d-tiles
NKT = S // 128             # 16 sk tiles
SCALE = float(HD) ** -0.5
BF = ml_dtypes.bfloat16

# even hd indices first, then odd (RoPE pair trick)
EVEN_FIRST = np.concatenate([np.arange(0, HD, 2), np.arange(1, HD, 2)])

_CACHE = {}
PHASE_MARKS = []


def _mark(nc, label):
    nid = nc.next_id()   # consumes one id; records emission position
    PHASE_MARKS.append((nid, label))


def phase_of(inst_name):
    try:
        n = int(inst_name.split("-")[1])
    except Exception:
        return "?"
    lab = "init"
    for nid, l in PHASE_MARKS:
        if n >= nid:
            lab = l
        else:
            break
    return lab


def _build_nc():
    import concourse.bacc as bacc
    import concourse.mybir as mybir
    import concourse.tile as tile

    BF16 = mybir.dt.bfloat16
    F32 = mybir.dt.float32
    AF = mybir.ActivationFunctionType
    ALU = mybir.AluOpType

    nc = bacc.Bacc("TRN2", target_bir_lowering=False, debug=False, num_devices=NCORES)

    # ---- per-core external inputs ----
    xT = nc.dram_tensor("xT", [D, S], BF16, kind="ExternalInput").ap()
    wq = nc.dram_tensor("wq", [D, QW], BF16, kind="ExternalInput").ap()    # col-permuted
    wk = nc.dram_tensor("wk", [D, HD], BF16, kind="ExternalInput").ap()    # col-permuted
    wv = nc.dram_tensor("wv", [D, HD], BF16, kind="ExternalInput").ap()
    wo = nc.dram_tensor("wo", [D, QW], BF16, kind="ExternalInput").ap()    # col slice
    cosT = nc.dram_tensor("cosT", [HD // 2, S], BF16, kind="ExternalInput").ap()
    sinT = nc.dram_tensor("sinT", [HD // 2, S], BF16, kind="ExternalInput").ap()
    maskd = nc.dram_tensor("maskd", [S, CHUNK], F32, kind="ExternalInput").ap()  # diag blocks, prescaled
    ident = nc.dram_tensor("ident", [128, 128], BF16, kind="ExternalInput").ap()

    # ---- per-core external outputs ----
    yt_o = nc.dram_tensor("yt", [QW, S], F32, kind="ExternalOutput").ap()   # y^T rows [c*512, (c+1)*512)
    kT_o = nc.dram_tensor("kT", [HD, S], BF16, kind="ExternalOutput").ap()  # roped k^T (hd permuted)
    v_o = nc.dram_tensor("v", [S, HD], BF16, kind="ExternalOutput").ap()    # v natural

    with tile.TileContext(nc) as tc:
        with (
            tc.tile_pool(name="persist", bufs=1) as persist,
            tc.tile_pool(name="xt", bufs=5) as xtp,
            tc.tile_pool(name="qt", bufs=9) as qtp,
            tc.tile_pool(name="rope", bufs=2) as ropep,
            tc.tile_pool(name="vt", bufs=2) as vtp,
            tc.tile_pool(name="md", bufs=4) as mdp,
            tc.tile_pool(name="recb", bufs=2) as recbp,
            tc.tile_pool(name="on", bufs=3) as onp,
            tc.tile_pool(name="ys", bufs=2) as ysp,
            tc.tile_pool(name="pt", bufs=4) as ptp,
            tc.tile_pool(name="og", bufs=2) as ogp,
            tc.tile_pool(name="small", bufs=4) as small,
            tc.tile_pool(name="psum", bufs=8, space="PSUM") as psum,
            tc.tile_pool(name="dram", bufs=1, space="DRAM") as dram,
        ):
            # ---- persistent tiles ----
            idt = persist.tile([128, 128], BF16, tag="ident")
            nc.sync.dma_start(idt[:], ident[:])
            cos_sb = persist.tile([HD // 2, S], BF16, tag="cos")
            sin_sb = persist.tile([HD // 2, S], BF16, tag="sin")
            nc.sync.dma_start(cos_sb[:], cosT[:])
            nc.sync.dma_start(sin_sb[:], sinT[:])
            ones = persist.tile([128, 128], BF16, tag="ones")
            nc.vector.memset(ones[:], 1.0)
            ones_row = persist.tile([1, 128], BF16, tag="ones_row")
            nc.vector.memset(ones_row[:], 1.0)

            # resident weights, [128, NDT*cols] with d-tile-major columns.
            # Split per d-tile so the first matmuls don't wait for the
            # whole preload; wq/wk/wv first (stage P), wo last (stage W).
            wqh_sb = [persist.tile([128, NDT * HD], BF16, tag=f"wq{h}", name=f"wqh{h}")
                      for h in range(HPC)]
            wk_sb = persist.tile([128, NDT * HD], BF16, tag="wk")
            wv_sb = persist.tile([128, NDT * HD], BF16, tag="wv")
            wo_sb = persist.tile([128, NDT * QW], BF16, tag="wo")

            def load_wqh(h, split=False):
                if split:
                    hw_ = NDT // 2
                    for piece in range(2):
                        nc.sync.dma_start(
                            wqh_sb[h][:, piece * hw_ * HD:(piece + 1) * hw_ * HD]
                            .rearrange("p (d q) -> p d q", d=hw_),
                            wq[piece * hw_ * 128:(piece + 1) * hw_ * 128, h * HD:(h + 1) * HD]
                            .rearrange("(d p) q -> p d q", p=128))
                else:
                    nc.sync.dma_start(
                        wqh_sb[h][:].rearrange("p (d q) -> p d q", d=NDT),
                        wq[:, h * HD:(h + 1) * HD].rearrange("(d p) q -> p d q", p=128))

            def load_wkv():
                nc.sync.dma_start(wk_sb[:].rearrange("p (d q) -> p d q", d=NDT),
                                  wk.rearrange("(d p) q -> p d q", p=128))
                nc.sync.dma_start(wv_sb[:].rearrange("p (d q) -> p d q", d=NDT),
                                  wv.rearrange("(d p) q -> p d q", p=128))

            def load_wo():
                nc.sync.dma_start(wo_sb[:].rearrange("p (d q) -> p d q", d=NDT),
                                  wo.rearrange("(d p) q -> p d q", p=128))

            load_wqh(0, split=True)

            # persistent activations
            kTr = persist.tile([128, S], BF16, tag="kTr")
            vnat = persist.tile([128, S], BF16, tag="vnat")       # sk-tile t at cols [t*128, ..)

            ag_ins = []
            ag_outs = []
            qt_chunks = []
            md_chunks = []

            def rope(dst, dst_col, ps, j):
                """dst[:, dst_col:dst_col+CHUNK] = rope(ps) (bf16 out).

                ps: psum [128, CHUNK] f32, rows 0:64 = even pairs (x0),
                rows 64:128 = odd (x1)."""
                c_sl = cos_sb[:, j * CHUNK:(j + 1) * CHUNK]
                s_sl = sin_sb[:, j * CHUNK:(j + 1) * CHUNK]
                x0 = ps[0:64, :]
                x1 = ps[64:128, :]
                t0 = ropep.tile([64, CHUNK], F32, tag="rt0")
                t1 = ropep.tile([64, CHUNK], F32, tag="rt1")
                nc.vector.tensor_tensor(t0[:], x0, c_sl, op=ALU.mult)
                nc.vector.tensor_tensor(t1[:], x1, s_sl, op=ALU.mult)
                nc.vector.tensor_tensor(dst[0:64, dst_col:dst_col + CHUNK], t0[:], t1[:], op=ALU.subtract)
                t2 = ropep.tile([64, CHUNK], F32, tag="rt0")
                t3 = ropep.tile([64, CHUNK], F32, tag="rt1")
                nc.vector.tensor_tensor(t2[:], x0, s_sl, op=ALU.mult)
                nc.vector.tensor_tensor(t3[:], x1, c_sl, op=ALU.mult)
                nc.vector.tensor_tensor(dst[64:128, dst_col:dst_col + CHUNK], t2[:], t3[:], op=ALU.add)

            def load_x(j):
                c0 = j * CHUNK
                xsup = []
                for g in range(NDT // 8):
                    xs = xtp.tile([128, 8 * CHUNK], BF16, tag="xt", name=f"xs{j}_{g}")
                    nc.sync.dma_start(
                        xs[:].rearrange("p (d q) -> p d q", d=8),
                        xT[g * 1024:(g + 1) * 1024, c0:c0 + CHUNK].rearrange("(d p) q -> p d q", p=128))
                    xsup.append(xs)
                return xsup

            def stage_P(j, xsup, first=False):
                _mark(nc, f"P{j}")
                c0 = j * CHUNK

                def xts(d):
                    return xsup[d // 8][:, (d % 8) * CHUNK:(d % 8 + 1) * CHUNK]

                mds = []
                for jj in range(4):
                    md = mdp.tile([128, CHUNK], F32, tag="md", name=f"md{j}_{jj}")
                    nc.sync.dma_start(md[:], maskd[c0 + jj * 128: c0 + (jj + 1) * 128, :])
                    mds.append(md)
                md_chunks.append(mds)
                if first:
                    for _h in range(1, HPC):
                        load_wqh(_h)
                    load_wkv()
                    load_wo()
                qts = []
                for h in range(HPC):
                    qps = psum.tile([128, CHUNK], F32, tag="m", name=f"qps{j}_{h}")
                    for d in range(NDT):
                        nc.tensor.matmul(qps[:], wqh_sb[h][:, d * HD:(d + 1) * HD],
                                         xts(d), start=(d == 0), stop=(d == NDT - 1))
                    qt = qtp.tile([128, CHUNK], BF16, tag="qt", name=f"qt{j}_{h}")
                    rope(qt, 0, qps, j)
                    qts.append(qt)
                qt_chunks.append(qts)
                kps = psum.tile([128, CHUNK], F32, tag="m")
                vps = psum.tile([128, CHUNK], F32, tag="m")
                for d in range(NDT):
                    st_, sp_ = (d == 0), (d == NDT - 1)
                    nc.tensor.matmul(kps[:], wk_sb[:, d * HD:(d + 1) * HD], xts(d),
                                     start=st_, stop=sp_)
                    nc.tensor.matmul(vps[:], wv_sb[:, d * HD:(d + 1) * HD], xts(d),
                                     start=st_, stop=sp_)
                rope(kTr, c0, kps, j)
                nc.sync.dma_start(kT_o[:, c0:c0 + CHUNK], kTr[:, c0:c0 + CHUNK])
                vt = vtp.tile([128, CHUNK], BF16, tag="vt")
                nc.vector.tensor_copy(vt[:], vps[:])
                for tt in range(CHUNK // 128):
                    tp = psum.tile([128, 128], BF16, tag="m", name=f"tp{j}_{tt}")
                    nc.tensor.transpose(tp[:], vt[:, tt * 128:(tt + 1) * 128], idt[:])
                    t_glob = j * (CHUNK // 128) + tt
                    nc.scalar.activation(vnat[:, t_glob * 128:(t_glob + 1) * 128], tp[:], AF.Copy)
                    nc.sync.dma_start(v_o[t_glob * 128:(t_glob + 1) * 128, :],
                                      vnat[:, t_glob * 128:(t_glob + 1) * 128])

            def stage_A(j):
                _mark(nc, f"A{j}")
                c0 = j * CHUNK
                nblk = 4 * (j + 1)
                mds = md_chunks[j]
                ag_in = dram.tile([QW, CHUNK], BF16, tag=f"agin{j}")

                def do_norm(den, ov, h):
                    rec = small.tile([1, CHUNK], F32, tag="rec")
                    nc.vector.reciprocal(rec[:], den[0:1, :])
                    rec_d = dram.tile([1, CHUNK], F32, tag=f"recd{h % 2}", name=f"recd{j}_{h}")
                    nc.sync.dma_start(rec_d[:], rec[:])
                    recb = recbp.tile([128, CHUNK], F32, tag="recb")
                    nc.sync.dma_start(recb[:], rec_d[0:1, :].partition_broadcast(128))
                    onorm = onp.tile([128, CHUNK], BF16, tag="onorm")
                    nc.vector.tensor_tensor(onorm[:], ov[:], recb[:], op=ALU.mult)
                    nc.sync.dma_start(ag_in[h * 128:(h + 1) * 128, :], onorm[:])

                parts = []

                def issue_half(pi):
                    ag_o = dram.tile([NCORES * 2 * HD, CHUNK], BF16, tag=f"agout{j}_{pi}",
                                     addr_space="Shared", name=f"agout{j}_{pi}")
                    nc.gpsimd.collective_compute(
                        "AllGather", ALU.bypass,
                        ins=[ag_in[pi * 2 * HD:(pi + 1) * 2 * HD, :].opt()], outs=[ag_o.opt()],
                        replica_groups=[list(range(NCORES))],
                    )
                    parts.append(ag_o)

                pending = []
                for h in range(HPC):
                    q_sl = qt_chunks[j][h][:]
                    den = psum.tile([128, CHUNK], F32, tag="m")
                    ov = psum.tile([128, CHUNK], F32, tag="m")
                    queue = []      # (t, pt) awaiting den/ov matmuls (depth 2)
                    for t in range(nblk):
                        st = psum.tile([128, CHUNK], F32, tag="m")
                        nc.tensor.matmul(st[:], kTr[:, t * 128:(t + 1) * 128], q_sl,
                                         start=True, stop=True)
                        if t == 0 and pending:
                            do_norm(*pending.pop(0))
                        if t == 1 and h == 2 and j == NCH - 1:
                            issue_half(0)
                        if t >= nblk - 4:
                            nc.vector.tensor_tensor(st[:], st[:], mds[t - (nblk - 4)][:], op=ALU.add)
                        pt = ptp.tile([128, CHUNK], BF16, tag="pt")
                        nc.scalar.activation(pt[:], st[:], AF.Exp, scale=SCALE)
                        queue.append((t, pt))
                        if len(queue) > 2:
                            tp_, pv = queue.pop(0)
                            nc.tensor.matmul(den[:], ones[:], pv[:],
                                             start=(tp_ == 0), stop=False)
                            nc.tensor.matmul(ov[:], vnat[:, tp_ * 128:(tp_ + 1) * 128], pv[:],
                                             start=(tp_ == 0), stop=False)
                    while queue:
                        tp_, pv = queue.pop(0)
                        nc.tensor.matmul(den[:], ones[:], pv[:],
                                         start=(tp_ == 0), stop=(tp_ == nblk - 1))
                        nc.tensor.matmul(ov[:], vnat[:, tp_ * 128:(tp_ + 1) * 128], pv[:],
                                         start=(tp_ == 0), stop=(tp_ == nblk - 1))
                    pending.append((den, ov, h))
                while pending:
                    do_norm(*pending.pop(0))
                if j == NCH - 1:
                    issue_half(1)
                else:
                    ag_o = dram.tile([H * HD, CHUNK], BF16, tag=f"agout{j}",
                                     addr_space="Shared", name=f"agoutw{j}")
                    nc.gpsimd.collective_compute(
                        "AllGather", ALU.bypass,
                        ins=[ag_in.opt()], outs=[ag_o.opt()],
                        replica_groups=[list(range(NCORES))],
                    )
                    parts.append(ag_o)
                ag_outs.append(tuple(parts))
                ag_ins.append(ag_in)

            def stage_W(j):
                _mark(nc, f"W{j}")
                c0 = j * CHUNK
                ag_parts = ag_outs[j]
                yps = [psum.tile([128, CHUNK], F32, tag="m", name=f"yps{j}_{_d}") for _d in range(QW // 128)]
                # whole AG: rows are e = head*128 directly.
                # split AG part pi: heads {4r+2pi, 4r+2pi+1} at rows r*256 -> e = 4r+2pi+hl
                ogsup = []
                es_all = []
                if len(ag_parts) == 1:
                    for g in range(NDT // 8):
                        ogs = ogp.tile([128, 8 * CHUNK], BF16, tag="og", name=f"og{j}_{g}")
                        nc.sync.dma_start(
                            ogs[:].rearrange("p (d q) -> p d q", d=8),
                            ag_parts[0][g * 1024:(g + 1) * 1024, :].rearrange("(d p) q -> p d q", p=128))
                        ogsup.append(ogs)
                        es_all.append([8 * g + i for i in range(8)])
                else:
                    for pi, ag in enumerate(ag_parts):
                        for g in range(2):
                            ogs = ogp.tile([128, 8 * CHUNK], BF16, tag="og", name=f"og{j}_{pi}_{g}")
                            nc.sync.dma_start(
                                ogs[:].rearrange("p (d q) -> p d q", d=8),
                                ag[g * 1024:(g + 1) * 1024, :].rearrange("(d p) q -> p d q", p=128))
                            ogsup.append(ogs)
                            rs = [4 * (4 * g + rr) + 2 * pi + hl for rr in range(4) for hl in range(2)]
                            es_all.append(rs)
                for gi, ogs in enumerate(ogsup):
                    for i, e in enumerate(es_all[gi]):
                        og = ogs[:, i * CHUNK:(i + 1) * CHUNK]
                        first = (gi == 0 and i == 0)
                        last = (gi == len(ogsup) - 1 and i == len(es_all[gi]) - 1)
                        for dt_ in range(QW // 128):
                            nc.tensor.matmul(
                                yps[dt_][:], wo_sb[:, e * QW + dt_ * 128: e * QW + (dt_ + 1) * 128],
                                og, start=first, stop=last)
                for dt_ in range(QW // 128):
                    ysb = ysp.tile([128, CHUNK], F32, tag="ysb")
                    nc.scalar.activation(ysb[:], yps[dt_][:], AF.Copy)
                    nc.sync.dma_start(yt_o[dt_ * 128:(dt_ + 1) * 128, c0:c0 + CHUNK], ysb[:])

            # ---- emission: pipeline P/A, W trails by one chunk.
            # Weight preloads are staggered so early matmuls aren't stuck
            # behind the full 37MB preload in the DMA queues.
            warm_in = dram.tile([128, 16], BF16, tag="warm_in")
            warm_out = dram.tile([NCORES * 128, 16], BF16, tag="warm_out", addr_space="Shared")
            nc.gpsimd.collective_compute(
                "AllGather", ALU.bypass,
                ins=[warm_in.opt()], outs=[warm_out.opt()],
                replica_groups=[list(range(NCORES))],
            )
            xs_next = load_x(0)
            for j in range(NCH):
                stage_P(j, xs_next, first=(j == 0))
                if j + 1 < NCH:
                    xs_next = load_x(j + 1)
                stage_A(j)
                if j == 2:
                    stage_W(0)
                    stage_W(1)
            stage_W(2)
            stage_W(3)

    nc.compile()
    return nc


def _get_nc():
    if "nc" not in _CACHE:
        _CACHE["nc"] = _build_nc()
    return _CACHE["nc"]


def _prep_in_maps(x, freqs_cos, freqs_sin, mask, wq, wk, wv, wo):
    xT = np.ascontiguousarray(x.reshape(S, D).T).astype(BF)
    cosT = np.ascontiguousarray(freqs_cos.T).astype(BF)
    sinT = np.ascontiguousarray(freqs_sin.T).astype(BF)
    # stacked diagonal 512x512 blocks of mask^T, prescaled by 1/SCALE
    maskT = np.ascontiguousarray(mask.T).astype(np.float32)
    maskd = np.concatenate(
        [maskT[j * CHUNK:(j + 1) * CHUNK, j * CHUNK:(j + 1) * CHUNK] for j in range(NCH)],
        axis=0) * (1.0 / SCALE)
    maskd = maskd.astype(np.float32)
    ident = np.eye(128, dtype=BF)

    wqp = wq.reshape(D, H, HD)[:, :, EVEN_FIRST].reshape(D, H * HD).astype(BF)
    wkp = wk.reshape(D, HK, HD)[:, :, EVEN_FIRST].reshape(D, HK * HD).astype(BF)
    wv_ = wv.astype(BF)
    wo_ = wo.astype(BF)

    in_maps = []
    for c in range(NCORES):
        in_maps.append({
            "xT": xT,
            "wq": np.ascontiguousarray(wqp[:, c * QW:(c + 1) * QW]),
            "wk": np.ascontiguousarray(wkp[:, c * HD:(c + 1) * HD]),
            "wv": np.ascontiguousarray(wv_[:, c * HD:(c + 1) * HD]),
            "wo": np.ascontiguousarray(wo_[:, c * QW:(c + 1) * QW]),
            "cosT": cosT,
            "sinT": sinT,
            "maskd": maskd,
            "ident": ident,
        })
    return in_maps


def _assemble(results):
    # y: [1, S, D]
    ytT = np.concatenate([results[c]["yt"] for c in range(NCORES)], axis=0)  # [D, S]
    y = np.ascontiguousarray(ytT.T)[None].astype(np.float32)
    # cache_k: [1, S, HK, HD]; kT per core is [HD(perm), S]
    cache_k = np.empty((1, S, HK, HD), np.float32)
    cache_v = np.empty((1, S, HK, HD), np.float32)
    for c in range(NCORES):
        k_perm = results[c]["kT"].astype(np.float32).T     # [S, HD] permuted cols
        k_nat = np.empty((S, HD), np.float32)
        k_nat[:, EVEN_FIRST] = k_perm
        cache_k[0, :, c, :] = k_nat
        cache_v[0, :, c, :] = results[c]["v"].astype(np.float32)
    return y, cache_k, cache_v


def run_on_hw(in_maps, trace=False):
    from concourse.bass_utils import run_bass_kernel_spmd
    nc = _get_nc()
    return run_bass_kernel_spmd(nc, in_maps, core_ids=list(range(NCORES)), trace=trace)


def kernel(x, freqs_cos, freqs_sin, mask, input_idexes, cache_k, cache_v, wq, wk, wv, wo):
    idx = np.asarray(input_idexes)
    assert np.array_equal(idx, np.arange(S)), "kernel assumes identity cache scatter"
    in_maps = _prep_in_maps(np.asarray(x), np.asarray(freqs_cos), np.asarray(freqs_sin),
                            np.asarray(mask), np.asarray(wq), np.asarray(wk),
                            np.asarray(wv), np.asarray(wo))
    res = run_on_hw(in_maps, trace=False)
    return _assemble(res.results)


# revision 31
# speedup vs baseline: 1.0616x; 1.0108x over previous
"""Trainium2 8-core tensor-parallel GQA attention kernel (Bass/Tile).

Problem: B=1, S=2048, D=4096, H=32 query heads, Hk=8 kv heads, hd=128,
RoPE + causal mask + KV-cache identity scatter + output projection.

Sharding (8 cores): head-parallel tensor parallel.
  - core c: query heads [4c..4c+4), kv head c
  - wq/wk/wv column-sharded, wo column-sharded (AllGather of attention
    outputs instead of AllReduce of wo partials: the wo matmul performs
    the cross-head reduction locally after an AllGather of O^T, which
    moves 8x fewer bytes than an AllReduce of wo partials).

Layout: activations are kept transposed ("feature-major") on chip:
  xT [D, S], qT/kT [hd, S], scores^T [sk, sq] so that softmax's
  key-reduction maps to a ones-matmul and no transposes are needed in
  the attention inner loop.  RoPE's even/odd pairs are made contiguous
  by permuting the columns of wq/wk host-side (even hd indices first);
  the permutation cancels in q.k and is undone host-side for cache_k.

Compute dtype: bf16 (fp32 PSUM accumulation, fp32 softmax denominators).
"""
import numpy as np
import ml_dtypes

# ---- problem constants (hardcoded per spec) ----
S = 2048
D = 4096
H = 32
HK = 8
HD = 128
NCORES = 8
HPC = H // NCORES          # 4 query heads per core
QW = HPC * HD              # 512 q/wo columns per core
CHUNK = 512                # sq chunk
NCH = S // CHUNK           # 4 chunks
NDT = D // 128             # 32 d-tiles
NKT = S // 128             # 16 sk tiles
SCALE = float(HD) ** -0.5
BF = ml_dtypes.bfloat16

# even hd indices first, then odd (RoPE pair trick)
EVEN_FIRST = np.concatenate([np.arange(0, HD, 2), np.arange(1, HD, 2)])

_CACHE = {}
PHASE_MARKS = []


def _mark(nc, label):
    nid = nc.next_id()   # consumes one id; records emission position
    PHASE_MARKS.append((nid, label))


def phase_of(inst_name):
    try:
        n = int(inst_name.split("-")[1])
    except Exception:
        return "?"
    lab = "init"
    for nid, l in PHASE_MARKS:
        if n >= nid:
            lab = l
        else:
            break
    return lab


def _build_nc():
    import concourse.bacc as bacc
    import concourse.mybir as mybir
    import concourse.tile as tile

    BF16 = mybir.dt.bfloat16
    F32 = mybir.dt.float32
    AF = mybir.ActivationFunctionType
    ALU = mybir.AluOpType

    nc = bacc.Bacc("TRN2", target_bir_lowering=False, debug=False, num_devices=NCORES)

    # ---- per-core external inputs ----
    xT = nc.dram_tensor("xT", [D, S], BF16, kind="ExternalInput").ap()
    wq = nc.dram_tensor("wq", [D, QW], BF16, kind="ExternalInput").ap()    # col-permuted
    wk = nc.dram_tensor("wk", [D, HD], BF16, kind="ExternalInput").ap()    # col-permuted
    wv = nc.dram_tensor("wv", [D, HD], BF16, kind="ExternalInput").ap()
    wo = nc.dram_tensor("wo", [D, QW], BF16, kind="ExternalInput").ap()    # col slice
    cosT = nc.dram_tensor("cosT", [HD // 2, S], BF16, kind="ExternalInput").ap()
    sinT = nc.dram_tensor("sinT", [HD // 2, S], BF16, kind="ExternalInput").ap()
    maskd = nc.dram_tensor("maskd", [S, CHUNK], F32, kind="ExternalInput").ap()  # diag blocks, prescaled
    ident = nc.dram_tensor("ident", [128, 128], BF16, kind="ExternalInput").ap()

    # ---- per-core external outputs ----
    yt_o = nc.dram_tensor("yt", [QW, S], F32, kind="ExternalOutput").ap()   # y^T rows [c*512, (c+1)*512)
    kT_o = nc.dram_tensor("kT", [HD, S], BF16, kind="ExternalOutput").ap()  # roped k^T (hd permuted)
    v_o = nc.dram_tensor("v", [S, HD], BF16, kind="ExternalOutput").ap()    # v natural

    with tile.TileContext(nc) as tc:
        with (
            tc.tile_pool(name="persist", bufs=1) as persist,
            tc.tile_pool(name="xt", bufs=5) as xtp,
            tc.tile_pool(name="qt", bufs=9) as qtp,
            tc.tile_pool(name="rope", bufs=2) as ropep,
            tc.tile_pool(name="vt", bufs=2) as vtp,
            tc.tile_pool(name="md", bufs=4) as mdp,
            tc.tile_pool(name="recb", bufs=2) as recbp,
            tc.tile_pool(name="on", bufs=3) as onp,
            tc.tile_pool(name="ys", bufs=2) as ysp,
            tc.tile_pool(name="pt", bufs=4) as ptp,
            tc.tile_pool(name="og", bufs=2) as ogp,
            tc.tile_pool(name="small", bufs=4) as small,
            tc.tile_pool(name="psum", bufs=8, space="PSUM") as psum,
            tc.tile_pool(name="dram", bufs=1, space="DRAM") as dram,
        ):
            # ---- persistent tiles ----
            idt = persist.tile([128, 128], BF16, tag="ident")
            nc.sync.dma_start(idt[:], ident[:])
            cos_sb = persist.tile([HD // 2, S], BF16, tag="cos")
            sin_sb = persist.tile([HD // 2, S], BF16, tag="sin")
            nc.sync.dma_start(cos_sb[:], cosT[:])
            nc.sync.dma_start(sin_sb[:], sinT[:])
            ones = persist.tile([128, 128], BF16, tag="ones")
            nc.vector.memset(ones[:], 1.0)
            ones_row = persist.tile([1, 128], BF16, tag="ones_row")
            nc.vector.memset(ones_row[:], 1.0)

            # resident weights, [128, NDT*cols] with d-tile-major columns.
            # Split per d-tile so the first matmuls don't wait for the
            # whole preload; wq/wk/wv first (stage P), wo last (stage W).
            wqh_sb = [persist.tile([128, NDT * HD], BF16, tag=f"wq{h}", name=f"wqh{h}")
                      for h in range(HPC)]
            wk_sb = persist.tile([128, NDT * HD], BF16, tag="wk")
            wv_sb = persist.tile([128, NDT * HD], BF16, tag="wv")
            wo_sb = persist.tile([128, NDT * QW], BF16, tag="wo")

            def load_wqh(h, split=False):
                if split:
                    hw_ = NDT // 4
                    for piece in range(4):
                        nc.sync.dma_start(
                            wqh_sb[h][:, piece * hw_ * HD:(piece + 1) * hw_ * HD]
                            .rearrange("p (d q) -> p d q", d=hw_),
                            wq[piece * hw_ * 128:(piece + 1) * hw_ * 128, h * HD:(h + 1) * HD]
                            .rearrange("(d p) q -> p d q", p=128))
                else:
                    nc.sync.dma_start(
                        wqh_sb[h][:].rearrange("p (d q) -> p d q", d=NDT),
                        wq[:, h * HD:(h + 1) * HD].rearrange("(d p) q -> p d q", p=128))

            def load_wkv():
                nc.sync.dma_start(wk_sb[:].rearrange("p (d q) -> p d q", d=NDT),
                                  wk.rearrange("(d p) q -> p d q", p=128))
                nc.sync.dma_start(wv_sb[:].rearrange("p (d q) -> p d q", d=NDT),
                                  wv.rearrange("(d p) q -> p d q", p=128))

            def load_wo():
                nc.sync.dma_start(wo_sb[:].rearrange("p (d q) -> p d q", d=NDT),
                                  wo.rearrange("(d p) q -> p d q", p=128))

            load_wqh(0, split=True)

            # persistent activations
            kTr = persist.tile([128, S], BF16, tag="kTr")
            vnat = persist.tile([128, S], BF16, tag="vnat")       # sk-tile t at cols [t*128, ..)

            ag_ins = []
            ag_outs = []
            qt_chunks = []
            md_chunks = []

            def rope(dst, dst_col, ps, j):
                """dst[:, dst_col:dst_col+CHUNK] = rope(ps) (bf16 out).

                ps: psum [128, CHUNK] f32, rows 0:64 = even pairs (x0),
                rows 64:128 = odd (x1)."""
                c_sl = cos_sb[:, j * CHUNK:(j + 1) * CHUNK]
                s_sl = sin_sb[:, j * CHUNK:(j + 1) * CHUNK]
                x0 = ps[0:64, :]
                x1 = ps[64:128, :]
                t0 = ropep.tile([64, CHUNK], F32, tag="rt0")
                t1 = ropep.tile([64, CHUNK], F32, tag="rt1")
                nc.vector.tensor_tensor(t0[:], x0, c_sl, op=ALU.mult)
                nc.vector.tensor_tensor(t1[:], x1, s_sl, op=ALU.mult)
                nc.vector.tensor_tensor(dst[0:64, dst_col:dst_col + CHUNK], t0[:], t1[:], op=ALU.subtract)
                t2 = ropep.tile([64, CHUNK], F32, tag="rt0")
                t3 = ropep.tile([64, CHUNK], F32, tag="rt1")
                nc.vector.tensor_tensor(t2[:], x0, s_sl, op=ALU.mult)
                nc.vector.tensor_tensor(t3[:], x1, c_sl, op=ALU.mult)
                nc.vector.tensor_tensor(dst[64:128, dst_col:dst_col + CHUNK], t2[:], t3[:], op=ALU.add)

            def load_x(j):
                c0 = j * CHUNK
                xsup = []
                for g in range(NDT // 8):
                    xs = xtp.tile([128, 8 * CHUNK], BF16, tag="xt", name=f"xs{j}_{g}")
                    nc.sync.dma_start(
                        xs[:].rearrange("p (d q) -> p d q", d=8),
                        xT[g * 1024:(g + 1) * 1024, c0:c0 + CHUNK].rearrange("(d p) q -> p d q", p=128))
                    xsup.append(xs)
                return xsup

            def stage_P(j, xsup, first=False):
                _mark(nc, f"P{j}")
                c0 = j * CHUNK

                def xts(d):
                    return xsup[d // 8][:, (d % 8) * CHUNK:(d % 8 + 1) * CHUNK]

                mds = []
                for jj in range(4):
                    md = mdp.tile([128, CHUNK], F32, tag="md", name=f"md{j}_{jj}")
                    nc.sync.dma_start(md[:], maskd[c0 + jj * 128: c0 + (jj + 1) * 128, :])
                    mds.append(md)
                md_chunks.append(mds)
                if first:
                    for _h in range(1, HPC):
                        load_wqh(_h)
                    load_wkv()
                    load_wo()
                qts = []
                for h in range(HPC):
                    qps = psum.tile([128, CHUNK], F32, tag="m", name=f"qps{j}_{h}")
                    for d in range(NDT):
                        nc.tensor.matmul(qps[:], wqh_sb[h][:, d * HD:(d + 1) * HD],
                                         xts(d), start=(d == 0), stop=(d == NDT - 1))
                    qt = qtp.tile([128, CHUNK], BF16, tag="qt", name=f"qt{j}_{h}")
                    rope(qt, 0, qps, j)
                    qts.append(qt)
                qt_chunks.append(qts)
                kps = psum.tile([128, CHUNK], F32, tag="m")
                vps = psum.tile([128, CHUNK], F32, tag="m")
                for d in range(NDT):
                    st_, sp_ = (d == 0), (d == NDT - 1)
                    nc.tensor.matmul(kps[:], wk_sb[:, d * HD:(d + 1) * HD], xts(d),
                                     start=st_, stop=sp_)
                    nc.tensor.matmul(vps[:], wv_sb[:, d * HD:(d + 1) * HD], xts(d),
                                     start=st_, stop=sp_)
                rope(kTr, c0, kps, j)
                nc.sync.dma_start(kT_o[:, c0:c0 + CHUNK], kTr[:, c0:c0 + CHUNK])
                vt = vtp.tile([128, CHUNK], BF16, tag="vt")
                nc.vector.tensor_copy(vt[:], vps[:])
                for tt in range(CHUNK // 128):
                    tp = psum.tile([128, 128], BF16, tag="m", name=f"tp{j}_{tt}")
                    nc.tensor.transpose(tp[:], vt[:, tt * 128:(tt + 1) * 128], idt[:])
                    t_glob = j * (CHUNK // 128) + tt
                    nc.scalar.activation(vnat[:, t_glob * 128:(t_glob + 1) * 128], tp[:], AF.Copy)
                    nc.sync.dma_start(v_o[t_glob * 128:(t_glob + 1) * 128, :],
                                      vnat[:, t_glob * 128:(t_glob + 1) * 128])

            def stage_A(j):
                _mark(nc, f"A{j}")
                c0 = j * CHUNK
                nblk = 4 * (j + 1)
                mds = md_chunks[j]
                ag_in = dram.tile([QW, CHUNK], BF16, tag=f"agin{j}")

                def do_norm(den, ov, h):
                    rec = small.tile([1, CHUNK], F32, tag="rec")
                    nc.vector.reciprocal(rec[:], den[0:1, :])
                    rec_d = dram.tile([1, CHUNK], F32, tag=f"recd{h % 2}", name=f"recd{j}_{h}")
                    nc.sync.dma_start(rec_d[:], rec[:])
                    recb = recbp.tile([128, CHUNK], F32, tag="recb")
                    nc.sync.dma_start(recb[:], rec_d[0:1, :].partition_broadcast(128))
                    onorm = onp.tile([128, CHUNK], BF16, tag="onorm")
                    nc.vector.tensor_tensor(onorm[:], ov[:], recb[:], op=ALU.mult)
                    nc.sync.dma_start(ag_in[h * 128:(h + 1) * 128, :], onorm[:])

                parts = []

                def issue_half(pi):
                    ag_o = dram.tile([NCORES * 2 * HD, CHUNK], BF16, tag=f"agout{j}_{pi}",
                                     addr_space="Shared", name=f"agout{j}_{pi}")
                    nc.gpsimd.collective_compute(
                        "AllGather", ALU.bypass,
                        ins=[ag_in[pi * 2 * HD:(pi + 1) * 2 * HD, :].opt()], outs=[ag_o.opt()],
                        replica_groups=[list(range(NCORES))],
                    )
                    parts.append(ag_o)

                pending = []
                for h in range(HPC):
                    q_sl = qt_chunks[j][h][:]
                    den = psum.tile([128, CHUNK], F32, tag="m")
                    ov = psum.tile([128, CHUNK], F32, tag="m")
                    queue = []      # (t, pt) awaiting den/ov matmuls (depth 2)
                    for t in range(nblk):
                        st = psum.tile([128, CHUNK], F32, tag="m")
                        nc.tensor.matmul(st[:], kTr[:, t * 128:(t + 1) * 128], q_sl,
                                         start=True, stop=True)
                        if t == 0 and pending:
                            do_norm(*pending.pop(0))
                        if t == 1 and h == 2 and j == NCH - 1:
                            issue_half(0)
                        if t >= nblk - 4:
                            nc.vector.tensor_tensor(st[:], st[:], mds[t - (nblk - 4)][:], op=ALU.add)
                        pt = ptp.tile([128, CHUNK], BF16, tag="pt")
                        nc.scalar.activation(pt[:], st[:], AF.Exp, scale=SCALE)
                        queue.append((t, pt))
                        if len(queue) > 2:
                            tp_, pv = queue.pop(0)
                            nc.tensor.matmul(den[:], ones[:], pv[:],
                                             start=(tp_ == 0), stop=False)
                            nc.tensor.matmul(ov[:], vnat[:, tp_ * 128:(tp_ + 1) * 128], pv[:],
                                             start=(tp_ == 0), stop=False)
                    while queue:
                        tp_, pv = queue.pop(0)
                        nc.tensor.matmul(den[:], ones[:], pv[:],
                                         start=(tp_ == 0), stop=(tp_ == nblk - 1))
                        nc.tensor.matmul(ov[:], vnat[:, tp_ * 128:(tp_ + 1) * 128], pv[:],
                                         start=(tp_ == 0), stop=(tp_ == nblk - 1))
                    pending.append((den, ov, h))
                while pending:
                    do_norm(*pending.pop(0))
                if j == NCH - 1:
                    issue_half(1)
                else:
                    ag_o = dram.tile([H * HD, CHUNK], BF16, tag=f"agout{j}",
                                     addr_space="Shared", name=f"agoutw{j}")
                    nc.gpsimd.collective_compute(
                        "AllGather", ALU.bypass,
                        ins=[ag_in.opt()], outs=[ag_o.opt()],
                        replica_groups=[list(range(NCORES))],
                    )
                    parts.append(ag_o)
                ag_outs.append(tuple(parts))
                ag_ins.append(ag_in)

            def stage_W(j):
                _mark(nc, f"W{j}")
                c0 = j * CHUNK
                ag_parts = ag_outs[j]
                yps = [psum.tile([128, CHUNK], F32, tag="m", name=f"yps{j}_{_d}") for _d in range(QW // 128)]
                # whole AG: rows are e = head*128 directly.
                # split AG part pi: heads {4r+2pi, 4r+2pi+1} at rows r*256 -> e = 4r+2pi+hl
                ogsup = []
                es_all = []
                if len(ag_parts) == 1:
                    for g in range(NDT // 8):
                        ogs = ogp.tile([128, 8 * CHUNK], BF16, tag="og", name=f"og{j}_{g}")
                        nc.sync.dma_start(
                            ogs[:].rearrange("p (d q) -> p d q", d=8),
                            ag_parts[0][g * 1024:(g + 1) * 1024, :].rearrange("(d p) q -> p d q", p=128))
                        ogsup.append(ogs)
                        es_all.append([8 * g + i for i in range(8)])
                else:
                    for pi, ag in enumerate(ag_parts):
                        for g in range(2):
                            ogs = ogp.tile([128, 8 * CHUNK], BF16, tag="og", name=f"og{j}_{pi}_{g}")
                            nc.sync.dma_start(
                                ogs[:].rearrange("p (d q) -> p d q", d=8),
                                ag[g * 1024:(g + 1) * 1024, :].rearrange("(d p) q -> p d q", p=128))
                            ogsup.append(ogs)
                            rs = [4 * (4 * g + rr) + 2 * pi + hl for rr in range(4) for hl in range(2)]
                            es_all.append(rs)
                for gi, ogs in enumerate(ogsup):
                    for i, e in enumerate(es_all[gi]):
                        og = ogs[:, i * CHUNK:(i + 1) * CHUNK]
                        first = (gi == 0 and i == 0)
                        last = (gi == len(ogsup) - 1 and i == len(es_all[gi]) - 1)
                        for dt_ in range(QW // 128):
                            nc.tensor.matmul(
                                yps[dt_][:], wo_sb[:, e * QW + dt_ * 128: e * QW + (dt_ + 1) * 128],
                                og, start=first, stop=last)
                for dt_ in range(QW // 128):
                    ysb = ysp.tile([128, CHUNK], F32, tag="ysb")
                    nc.scalar.activation(ysb[:], yps[dt_][:], AF.Copy)
                    nc.sync.dma_start(yt_o[dt_ * 128:(dt_ + 1) * 128, c0:c0 + CHUNK], ysb[:])

            # ---- emission: pipeline P/A, W trails by one chunk.
            # Weight preloads are staggered so early matmuls aren't stuck
            # behind the full 37MB preload in the DMA queues.
            warm_in = dram.tile([128, 16], BF16, tag="warm_in")
            warm_out = dram.tile([NCORES * 128, 16], BF16, tag="warm_out", addr_space="Shared")
            nc.gpsimd.collective_compute(
                "AllGather", ALU.bypass,
                ins=[warm_in.opt()], outs=[warm_out.opt()],
                replica_groups=[list(range(NCORES))],
            )
            xs_next = load_x(0)
            for j in range(NCH):
                stage_P(j, xs_next, first=(j == 0))
                if j + 1 < NCH:
                    xs_next = load_x(j + 1)
                stage_A(j)
                if j == 2:
                    stage_W(0)
                    stage_W(1)
            stage_W(2)
            stage_W(3)

    nc.compile()
    return nc


def _get_nc():
    if "nc" not in _CACHE:
        _CACHE["nc"] = _build_nc()
    return _CACHE["nc"]


def _prep_in_maps(x, freqs_cos, freqs_sin, mask, wq, wk, wv, wo):
    xT = np.ascontiguousarray(x.reshape(S, D).T).astype(BF)
    cosT = np.ascontiguousarray(freqs_cos.T).astype(BF)
    sinT = np.ascontiguousarray(freqs_sin.T).astype(BF)
    # stacked diagonal 512x512 blocks of mask^T, prescaled by 1/SCALE
    maskT = np.ascontiguousarray(mask.T).astype(np.float32)
    maskd = np.concatenate(
        [maskT[j * CHUNK:(j + 1) * CHUNK, j * CHUNK:(j + 1) * CHUNK] for j in range(NCH)],
        axis=0) * (1.0 / SCALE)
    maskd = maskd.astype(np.float32)
    ident = np.eye(128, dtype=BF)

    wqp = wq.reshape(D, H, HD)[:, :, EVEN_FIRST].reshape(D, H * HD).astype(BF)
    wkp = wk.reshape(D, HK, HD)[:, :, EVEN_FIRST].reshape(D, HK * HD).astype(BF)
    wv_ = wv.astype(BF)
    wo_ = wo.astype(BF)

    in_maps = []
    for c in range(NCORES):
        in_maps.append({
            "xT": xT,
            "wq": np.ascontiguousarray(wqp[:, c * QW:(c + 1) * QW]),
            "wk": np.ascontiguousarray(wkp[:, c * HD:(c + 1) * HD]),
            "wv": np.ascontiguousarray(wv_[:, c * HD:(c + 1) * HD]),
            "wo": np.ascontiguousarray(wo_[:, c * QW:(c + 1) * QW]),
            "cosT": cosT,
            "sinT": sinT,
            "maskd": maskd,
            "ident": ident,
        })
    return in_maps


def _assemble(results):
    # y: [1, S, D]
    ytT = np.concatenate([results[c]["yt"] for c in range(NCORES)], axis=0)  # [D, S]
    y = np.ascontiguousarray(ytT.T)[None].astype(np.float32)
    # cache_k: [1, S, HK, HD]; kT per core is [HD(perm), S]
    cache_k = np.empty((1, S, HK, HD), np.float32)
    cache_v = np.empty((1, S, HK, HD), np.float32)
    for c in range(NCORES):
        k_perm = results[c]["kT"].astype(np.float32).T     # [S, HD] permuted cols
        k_nat = np.empty((S, HD), np.float32)
        k_nat[:, EVEN_FIRST] = k_perm
        cache_k[0, :, c, :] = k_nat
        cache_v[0, :, c, :] = results[c]["v"].astype(np.float32)
    return y, cache_k, cache_v


def run_on_hw(in_maps, trace=False):
    from concourse.bass_utils import run_bass_kernel_spmd
    nc = _get_nc()
    return run_bass_kernel_spmd(nc, in_maps, core_ids=list(range(NCORES)), trace=trace)


def kernel(x, freqs_cos, freqs_sin, mask, input_idexes, cache_k, cache_v, wq, wk, wv, wo):
    idx = np.asarray(input_idexes)
    assert np.array_equal(idx, np.arange(S)), "kernel assumes identity cache scatter"
    in_maps = _prep_in_maps(np.asarray(x), np.asarray(freqs_cos), np.asarray(freqs_sin),
                            np.asarray(mask), np.asarray(wq), np.asarray(wk),
                            np.asarray(wv), np.asarray(wo))
    res = run_on_hw(in_maps, trace=False)
    return _assemble(res.results)
